# revision 21
# baseline (speedup 1.0000x reference)
"""Multi-head causal self-attention on 8 Trainium2 NeuronCores (Bass/Tile).

Problem: y = proj(softmax(causal_mask(Q K^T / sqrt(D))) V) for B=2, T=2048,
C=1024, H=16 heads, D=64.

Sharding (tensor-parallel over heads, 8-way):
  - Core i owns heads {2i, 2i+1}: computes qT/kT/vT for its heads over both
    batches (full x, its 128-column slice of Wqkv) and runs causal attention
    per head fully on-core, producing *unnormalized* yT_local plus the
    reciprocal softmax denominators (head-dims on partitions, time free).
  - Two 8-way AllToAlls reshard head-split -> time-split. Each payload row
    group is [65, 512]: 64 rows of unnormalized y^T plus one row of
    reciprocal denominators. Core j ends with yT_full [128, 4096] (all 16
    heads) for time-slice j, normalizes it there (K=1 ones-matmul broadcast
    of the recip rows + one vector multiply per source core), and computes
    its [512, 1024] slice of y @ Wproj.
  - The host concatenates the 8 time-slices into [2, 2048, 1024].

All matmul operands are bf16 (PE streams 1 col/cycle vs 2-4 for
fp32/fp32r; measured on HW: fp32r 128x128 matmuls run at the full-fp32
4-cycle rate). Accumulation stays fp32 in PSUM. bf16 also halves HBM and
collective traffic. Error: 5.9e-3 measured vs 2e-2 tolerance.

Post-collective normalization: each (head, batch) ships unnormalized y^T
plus per-query reciprocal denominators (row 64 of the payload, per-j
reciprocals so the last j's chain into the collective trigger is short).
The receiver broadcasts recip rows across the 64 partitions of each head
half with one K=1 ones-matmul per source core and multiplies in place;
the h=0 half normalizes during the second collective's transfer window.

Scheduling notes (the tile scheduler reorders within engine queues by
priority, so program order alone does not pin queue order):
  - tc.tile_wait_until pins the finale (ytf/rsb pulls, normalize, proj)
    behind the attention phases; without it the scheduler hoists
    collective-gated DMAs/matmuls into the in-order PE/sync streams and
    stalls them ~50us on the first AllToAll.
  - The h=0 pulls ride the scalar queue (idle after its last exp); the
    sync queue is still draining attn(1,1) stage writes then.
  - Mesh AllToAlls: a quiet, synchronized A2A of this payload takes only
    ~9-12us (measured with a standalone microbench, even back-to-back),
    but the first collective after a long compute phase absorbs the
    PEER SKEW accumulated since the last sync point — per-core throttle
    variance over ~190us of compute spreads attention-end times by up
    to ~30us, so in-kernel A2As measure 17-90us run-to-run. A2A#1 hides
    under attn(1,*); A2A#2's exposure is skew + transfer.
  - The collective_compute instruction BLOCKS the gpsimd queue until the
    collective completes (doorbell + wait_ge(completion)). Anything
    queued on gpsimd behind A2A#1 — previously the h=1 denominator
    gathers — executes only after A2A#1 finishes, delaying every core's
    A2A#2 payload by ~10us and re-adding trigger skew. The gathers ride
    the scalar queue instead; best-case A2A#2 is now ~18us vs ~28us.
  - AllGather with a Shared-HBM output was tried and is catastrophically
    slow (~2.4s). A 4-core-group AllToAll (batch-DP x 4-way head-TP,
    which would pipeline 4 smaller collectives) is NOT supported: mesh
    collectives need >4 cores (concourse/replica_groups.py).
    remote_dma (SBUF->SBUF cross-core pushes) would kill the skew
    sensitivity entirely but requires 128-partition transfers and
    compile-time remote addresses, which collide under SPMD (all
    senders would write the same dest offset) — needs an indirect-DMA
    permute by per-core routing tables; not attempted.

Attention is computed transposed (S^T[k, q], keys on partitions): exp on
ScalarE straight out of PSUM. Causality is exact: S^T blocks strictly
above the diagonal are skipped; diagonal blocks use a restricted column
range plus a triangular multiplicative mask after exp. Blocks are
processed in 1024-wide pairs (full and diagonal alike) so one ACTIVATE
covers two blocks; gaps between paired diagonal blocks exp garbage that
no PV matmul ever reads. Chunks are software-pipelined TWO deep (S/exp
of chunks i+1 and i+2 issue before PV of chunk i): one ScalarE exp
(~1.3us/chunk) is LONGER than one S pair (~0.7us), so a one-deep
pipeline stalls the in-order PE queue ~0.5us per chunk and drops the
PE out of its high p-state (PE ramps 0.65->1.2->2.4GHz; sustained
8-core attention power-caps the PE at ~1.2GHz — matmul p50 427ns vs
215ns after an idle window — so PE COLUMN COUNT is the binding compute
currency). With lag-2 the PE runs 97-100% busy through attention.

PV streams V|ones as the MOVING operand (65 cols per 128-key block,
exp'd P block [128k x 128q] as stationary) instead of streaming 512
P-columns per block: PE cost is moving columns, so PV drops from 512
to <=260 col-cycles per block (~44us -> ~34us measured, more at capped
clock; LoadStationary is fully hidden even for 65-col streams). The
softmax denominator comes free as output COLUMN 64 (ones column of V).
Output lands as y[q, d|denom] per 128-q subtile in one PSUM bank (a
single accumulation group per slice: start=True zeroes the whole 2KB
zero-region and only ONE group may be pending per bank, so only the
first matmul starts and only the last stops); at slice end it is cast
to SBUF and transposed back to [d|denom, q] (four 128-col transposes)
for the unchanged A2A wire format. S is already column-optimal (128
cols per 128x128 block); qkv and proj are at their column floors.

Denominators per (h,b) are gathered compactly (slice j at partitions
4j) and hit ONE batched DVE reciprocal instead of four: reciprocal is
overhead-dominated (~1.1us regardless of size), so this frees ~13us of
DVE without lengthening the chain into the collective trigger.

DMA discipline: Wqkv arrives host-prepacked in SBUF layout so each
projection loads as one wide-row 2D DMA (v-slice first — it gates the
program's first matmul); the first x chunk loads per-ct so the first
accumulation group pipelines with DMA arrival; remaining bulk loads are
coalesced multi-tile transfers on the sync queue. Attention-tail DMAs
(a2a stage writes, recip rows) ride the sync queue in data-ready order;
denominator gathers sit on the (otherwise idle) GpSimd queue so the
collective triggers never wait behind descriptor pushes.
"""

import numpy as np
import ml_dtypes

import concourse.bass as bass
import concourse.mybir as mybir
import concourse.tile as tile
from concourse import bacc
from concourse import bass_utils

F32 = mybir.dt.float32
BF16 = mybir.dt.bfloat16
AF = mybir.ActivationFunctionType

B, T, C = 2, 2048, 1024
H, D = 16, 64
N_CORES = 8
HL = H // N_CORES        # heads per core = 2
NCT = C // 128           # contraction tiles = 8
NQ = T // 512            # q tiles per batch = 4
NK = T // 128            # k tiles per batch = 16
SCALE = 1.0 / float(np.sqrt(D))  # 0.125

_BUILD_CACHE = {}


def _drain(*gens):
    """Round-robin the generators until all are exhausted."""
    active = list(gens)
    while active:
        nxt = []
        for g in active:
            try:
                next(g)
                nxt.append(g)
            except StopIteration:
                pass
        active = nxt


def build_kernel(apply_pad_mask: bool):
    nc = bacc.Bacc(
        "TRN2", target_bir_lowering=False, debug=False, num_devices=N_CORES
    )
    xT = nc.dram_tensor("xT", [C, B * T], BF16, kind="ExternalInput").ap()
    wqkv = nc.dram_tensor("wqkv", [128, NCT * 3 * HL * D], BF16,
                          kind="ExternalInput").ap()
    wo = nc.dram_tensor("wo", [C, C], BF16, kind="ExternalInput").ap()
    tri = nc.dram_tensor("tri", [128, 128], F32, kind="ExternalInput").ap()
    ident = nc.dram_tensor("ident", [128, 128], BF16, kind="ExternalInput").ap()
    padk = nc.dram_tensor("padk", [128, B * NK], F32, kind="ExternalInput").ap()
    out = nc.dram_tensor("out", [512, C], BF16, kind="ExternalOutput").ap()

    with tile.TileContext(nc) as tc:
        with (
            tc.tile_pool(name="const", bufs=1) as constp,
            tc.tile_pool(name="qk", bufs=1) as qkp,
            tc.tile_pool(name="vv", bufs=1) as vvp,
            tc.tile_pool(name="xw", bufs=1) as xwp,
            tc.tile_pool(name="work", bufs=2) as wk,
            tc.tile_pool(name="ps_ss", bufs=2, space="PSUM") as ps_ss,
            tc.tile_pool(name="ps_main", bufs=2, space="PSUM") as ps_main,
            tc.tile_pool(name="ps_y", bufs=2, space="PSUM") as ps_y,
            tc.tile_pool(name="dram", bufs=1, space="DRAM") as dram,
        ):
            # ---------------- constants ----------------
            tri_sb = constp.tile([128, 128], F32, name="tri_sb")
            id_sb = constp.tile([128, 128], BF16, name="id_sb")
            # head-half selectors for the recip broadcast matmuls, present
            # at partition bases 0 and 64 (PE stationary/moving bases must
            # match and be 0/32/64)
            ones_h0 = constp.tile([65, 128], BF16, name="ones_h0")
            ones_h1 = constp.tile([65, 128], BF16, name="ones_h1")
            for r in (0, 64):
                nc.vector.memset(ones_h0[r:r + 1, 0:64], 1.0)
                nc.vector.memset(ones_h0[r:r + 1, 64:128], 0.0)
                nc.vector.memset(ones_h1[r:r + 1, 0:64], 0.0)
                nc.vector.memset(ones_h1[r:r + 1, 64:128], 1.0)
            onesc_f = constp.tile([128, HL], F32, name="onesc_f")
            nc.vector.memset(onesc_f[:], 1.0)
            onesc = constp.tile([128, HL], BF16, name="onesc")
            nc.gpsimd.tensor_copy(onesc[:], onesc_f[:])
            if apply_pad_mask:
                padk_sb = constp.tile([128, B * NK], F32, name="padk_sb")
                nc.sync.dma_start(padk_sb[:], padk[:])

            a2a_in = [dram.tile([N_CORES, 65, 512], BF16, name=f"a2a_in{h}")
                      for h in range(HL)]
            a2a_out = [dram.tile([N_CORES, 65, 512], BF16, name=f"a2a_out{h}")
                       for h in range(HL)]

            # big coalesced input tiles: slice ct lives at cols [ct*W,(ct+1)*W)
            # wqkv arrives host-prepacked in SBUF layout [p, which, ct, 128]
            # so each projection loads as one wide-row 2D DMA; the v-slice
            # gates the program's first matmul and heads the sync queue
            wqkv_sb = xwp.tile([128, NCT * 3 * HL * D], BF16, name="wqkv_sb")
            for which in (2, 1, 0):
                eng = nc.sync if which == 2 else nc.scalar
                eng.dma_start(
                    wqkv_sb[:, which * 1024:(which + 1) * 1024],
                    wqkv[:, which * 1024:(which + 1) * 1024],
                )
            xt_sb = xwp.tile([128, NCT * B * T], BF16, name="xt_sb")

            def xw(ct, which):
                return wqkv_sb[:, which * 1024 + ct * 128:
                               which * 1024 + (ct + 1) * 128]

            def xx(ct, b):
                return xt_sb[:, ct * B * T + b * T: ct * B * T + (b + 1) * T]

            qT = [None] * B
            kT = [None] * B
            V = [[None] * NK for _ in range(B)]

            def xload(b, c0, c1, eng=None):
                (eng or nc.sync).dma_start(
                    xt_sb[:].rearrange("p (c t) -> p c t", c=NCT)[:, :, b * T + c0:b * T + c1],
                    xT[:].rearrange("(c p) t -> p c t", c=NCT)[:, :, b * T + c0:b * T + c1],
                )

            def qkv_emit(b):
                """Projections for batch b: yields between schedulable
                chunks so the PE stream can interleave with attention."""
                if b == 0:
                    # first x chunk per-ct: the first matmul group's ct-loop
                    # pipelines with DMA arrival instead of waiting for one
                    # monolithic multi-tile transfer
                    for ct in range(NCT):
                        nc.sync.dma_start(
                            xt_sb[:, ct * B * T:ct * B * T + 512],
                            xT[ct * 128:(ct + 1) * 128, 0:512],
                        )
                    # constants ride behind the critical first loads
                    nc.sync.dma_start(tri_sb[:], tri[:])
                    nc.sync.dma_start(id_sb[:], ident[:])
                    # later x chunks push from the gpsimd queue in parallel
                    # with the per-ct pushes above — their transfers start
                    # ~5us earlier than queued behind 10 sync pushes
                    xload(0, 512, 1024, nc.gpsimd)
                    xload(0, 1024, T, nc.gpsimd)
                qT[b] = qkp.tile([128, T], BF16, name="qT", tag=f"qT{b}")
                kT[b] = qkp.tile([128, T], BF16, name="kT", tag=f"kT{b}")
                vT = qkp.tile([128, T], BF16, name="vT", tag="vT")
                dsts = {2: vT, 1: kT[b], 0: qT[b]}
                if b == 0:
                    # x streams in while qkv(0) runs: early groups only wait
                    # on 512 x-columns each, and low-column k/q groups fill
                    # the stalls before the later x chunks land
                    sched = [(2, (0,)), (2, (1,)), (1, (0, 1)), (2, (2, 3)),
                             (0, (0, 1)), (1, (2, 3)), (0, (2, 3))]
                else:
                    sched = [(2, (0, 1)), (2, (2, 3)), (1, (0, 1)),
                             (1, (2, 3)), (0, (0, 1)), (0, (2, 3))]
                for which, ns in sched:
                    dst = dsts[which]
                    if True:
                        p2 = [ps_main.tile([128, 512], F32, name="p_mm",
                                           tag="ps") for _ in ns]
                        for ct in range(NCT):
                            for ni, n in enumerate(ns):
                                nc.tensor.matmul(
                                    p2[ni][:],
                                    xw(ct, which),
                                    xx(ct, b)[:, n * 512:(n + 1) * 512],
                                    start=(ct == 0),
                                    stop=(ct == NCT - 1),
                                )
                        for ni, n in enumerate(ns):
                            nc.vector.tensor_copy(
                                dst[:, n * 512:(n + 1) * 512], p2[ni][:])
                            yield
                        if b == 0 and which == 2 and ns == (1,):
                            # batch-1 x arrives while batch-0 projects
                            xload(1, 0, T)
                for kt in range(NK):
                    v_sb = vvp.tile([128, HL * 65], BF16, name=f"V{b}_{kt}",
                                    tag=f"V{b}_{kt}")
                    pt = ps_main.tile([128, 128], BF16, name="p_tr", tag="ps")
                    nc.tensor.transpose(pt[:], vT[:, kt * 128:(kt + 1) * 128],
                                        id_sb[:])
                    v3 = v_sb[:].rearrange("p (h e) -> p h e", h=HL)
                    nc.gpsimd.tensor_copy(v3[:, :, 64], onesc[:])
                    nc.vector.tensor_copy(
                        v3[:, :, 0:64],
                        pt[:].rearrange("p (h e) -> p h e", h=HL),
                    )
                    V[b][kt] = v_sb
                    if kt % 4 == 3:
                        yield

            def attn_emit(h, b):
                """Attention for head-row h, batch b. Chunks are pairs of
                128-key blocks sharing one [128,1024] PSUM tile and one
                exp ACTIVATE; PV of chunk i issues after S/exp of chunks
                i+1 AND i+2 (two-deep software pipeline) so the ScalarE
                exp latency (~1.3us per chunk, longer than one S pair)
                never stalls the in-order PE queue and the PE stays in
                its high p-state."""
                h0 = h * 64
                # denominators packed compactly (j at rows 4j): one batched
                # DVE reciprocal per (h,b) instead of four — reciprocal is
                # overhead-dominated (~1.1us regardless of size)
                coll = wk.tile([16, 128], BF16, name="coll",
                               tag=f"coll{h}{b}", bufs=1)
                pending = []  # up to 2 of (p_sb, blocks, first_j, n_kt, stage_j)
                # py is single-buffered ([128,260] f32, 1 PSUM bank): it is
                # allocated lazily at the FIRST flush of each slice j, which
                # the lag-2 pipeline guarantees happens after the previous
                # slice's evacuation (yq copy) was emitted — allocating at
                # the j-loop top would rotate the buffer while the old
                # tile's reader was still unemitted.
                cur_py = [None]

                def flush(pend):
                    # PV streams V|ones as the MOVING operand (65 cols per
                    # 128-key block) with the exp'd P block as stationary,
                    # instead of streaming 512 P-columns per block: PE cost
                    # is moving columns, so PV drops from 512 to <=260
                    # col-cycles per block. The PE clock is power-capped at
                    # ~1.2GHz during sustained attention (matmul p50 427ns
                    # vs 215ns after an idle window), so column count is
                    # the binding currency (~20us saved). Output lands as
                    # y[q, d|denom] per 128-q subtile and is transposed
                    # back to [d|denom, q] at slice end (cheap: 128-col
                    # transposes, LoadStationary fully hidden).
                    p_sb, blocks, first_j, n_kt, stage_j = pend
                    if first_j:
                        cur_py[0] = ps_y.tile([128, 512], F32, name="p_y",
                                              tag="py", bufs=1)
                    py = cur_py[0]
                    # one PSUM accumulation group for the whole bank: a
                    # start=True zeroes the full 2KB zero-region and only
                    # one group may be pending per bank, so the first
                    # matmul (kt=0, sub=0) starts and the last
                    # (kt=n_kt-1, sub=3) stops; everything in between
                    # accumulates with both flags false.
                    for ci, (kt, off) in enumerate(blocks):
                        base = 512 * ci
                        for sub in range(off // 128, 4):
                            nc.tensor.matmul(
                                py[:, sub * 128:sub * 128 + 65],
                                p_sb[:, base + sub * 128:
                                     base + (sub + 1) * 128],
                                V[b][kt][:, h * 65:(h + 1) * 65],
                                start=(kt == 0 and sub == 0),
                                stop=(kt == n_kt - 1 and sub == 3),
                            )
                    if stage_j is not None:
                        # evacuate: PSUM [128q, 4x65] -> SBUF bf16 ->
                        # transpose each subtile back to [65, 128q] ->
                        # stage [65, 512] (rows 0:64 y^T, row 64 denoms)
                        jj = stage_j
                        m = b * NQ + jj
                        yq = wk.tile([128, 512], BF16, name="yq",
                                     tag="yq", bufs=2)
                        nc.vector.tensor_copy(yq[:], py[:])
                        tro = ps_y.tile([65, 512], BF16, name="tro",
                                        tag="trout", bufs=1)
                        for sub in range(4):
                            nc.tensor.transpose(
                                tro[0:65, sub * 128:(sub + 1) * 128],
                                yq[:, sub * 128:sub * 128 + 65],
                                id_sb[:],
                            )
                        stage = wk.tile([65, 512], BF16, name="stage",
                                        tag="stage", bufs=4)
                        nc.vector.tensor_copy(stage[:], tro[:])
                        nc.sync.dma_start(a2a_in[h][m, 0:64, :],
                                          stage[0:64, :])
                        # denominator gather rides the scalar queue, NOT
                        # gpsimd: the collective_compute instruction blocks
                        # the gpsimd queue until the collective completes,
                        # so h=1 gathers queued there would only run after
                        # A2A#1 finishes — delaying every core's A2A#2
                        # payload by ~10us and re-introducing trigger skew
                        # (A2A#2 then runs 28-50us instead of ~10us).
                        nc.scalar.dma_start(coll[4 * jj:4 * jj + 4, :],
                                            stage[64:65, :])

                for j in range(NQ):
                    q0 = j * 512
                    n_kt = 4 * j + 4
                    # chunks: 1024-wide pairs of (kt, col_offset) blocks
                    chunks = []
                    for kt in range(0, 4 * j, 2):
                        chunks.append(((kt, 0), (kt + 1, 0)))
                    for i in (0, 2):
                        chunks.append(((4 * j + i, 128 * i),
                                       (4 * j + i + 1, 128 * (i + 1))))
                    last_ci = len(chunks) - 1
                    for ci_chunk, blocks in enumerate(chunks):
                        pss = ps_ss.tile([128, 1024], F32, name="p_s",
                                         tag="pss")
                        lo = blocks[0][1]
                        # second block of a pair widens toward column 512 so
                        # one ACTIVATE can span it with no uninitialized
                        # PSUM gap — but only when the gap is cheaper than a
                        # second ACTIVATE (~0.4us): wide gaps (>128 cols)
                        # keep their exact range and get their own ACTIVATE.
                        # Widened above-diagonal columns are never read by PV.
                        off1 = blocks[1][1]
                        split_act = off1 > 128
                        for ci, (kt, off) in enumerate(blocks):
                            base = 512 * ci
                            s_off = off if (ci == 0 or split_act) else 0
                            nc.tensor.matmul(
                                pss[:, base + s_off:base + 512],
                                kT[b][h0:h0 + 64, kt * 128:(kt + 1) * 128],
                                qT[b][h0:h0 + 64, q0 + s_off:q0 + 512],
                                start=True,
                                stop=True,
                            )
                        p_sb = wk.tile([128, 1024], BF16, name="p_sb",
                                       tag="p_sb", bufs=3)
                        if split_act:
                            nc.scalar.activation(
                                p_sb[:, lo:512], pss[:, lo:512], AF.Exp,
                                scale=float(SCALE),
                            )
                            nc.scalar.activation(
                                p_sb[:, 512 + off1:1024],
                                pss[:, 512 + off1:1024], AF.Exp,
                                scale=float(SCALE),
                            )
                        else:
                            nc.scalar.activation(
                                p_sb[:, lo:1024], pss[:, lo:1024], AF.Exp,
                                scale=float(SCALE),
                            )
                        for ci, (kt, off) in enumerate(blocks):
                            base = 512 * ci
                            if kt >= 4 * j:
                                nc.vector.tensor_mul(
                                    p_sb[:, base + off:base + off + 128],
                                    p_sb[:, base + off:base + off + 128],
                                    tri_sb[:],
                                )
                            if apply_pad_mask:
                                nc.vector.tensor_scalar_mul(
                                    p_sb[:, base + off:base + 512],
                                    p_sb[:, base + off:base + 512],
                                    padk_sb[:, b * NK + kt:b * NK + kt + 1],
                                )
                        pending.append((p_sb, blocks, ci_chunk == 0, n_kt,
                                        j if ci_chunk == last_ci else None))
                        if len(pending) > 2:
                            flush(pending.pop(0))
                            yield
                while pending:
                    flush(pending.pop(0))
                    yield
                # one batched reciprocal: same latency as a per-j one
                # (overhead-dominated) so the chain from the last j's
                # denominators into the collective trigger is unchanged,
                # but the DVE does one op instead of four
                rcol = wk.tile([16, 128], BF16, name="rcol",
                               tag=f"rcol{h}{b}", bufs=1)
                with nc.allow_low_precision(reason="bf16 softmax denom"):
                    nc.vector.reciprocal(rcol[0:16, :], coll[0:16, :])
                for j in range(NQ):
                    m = b * NQ + j
                    nc.sync.dma_start(
                        a2a_in[h][m, 64, :].rearrange("(r c) -> r c", r=4),
                        rcol[4 * j:4 * j + 4, :],
                    )
                yield

            wo_sb = xwp.tile([128, NCT * C], BF16, name="wo_sb")
            ytf = xwp.tile([128, N_CORES * 512], BF16, name="ytf")
            rsb = [None] * HL

            def wo_emit():
                # prefetch Wproj during attn(0,1)
                nc.sync.dma_start(
                    wo_sb[:].rearrange("p (c k) -> p c k", c=NCT),
                    wo[:].rearrange("(c p) k -> p c k", c=NCT),
                )
                yield

            def ytf_emit(h):
                # pull the h half of yT_full + recip rows once A2A h lands.
                # The tiny recip pulls go first and ride the scalar queue
                # (idle once its exps are done): they gate the pb broadcast
                # matmuls, while the big ytf pull only gates the later
                # vector multiplies — the two queues push in parallel.
                rsb[h] = xwp.tile([65, 4 * 512], BF16, name=f"rsb{h}",
                                  tag=f"rsb{h}")
                for g in (0, 1):
                    nc.scalar.dma_start(
                        rsb[h][64 * g:64 * g + 1, :].rearrange(
                            "p (s t) -> p s t", s=4),
                        a2a_out[h][4 * g:4 * g + 4, 64:65, :].rearrange(
                            "s p t -> p s t"),
                    )
                if h == 0:
                    # off the critical path: one coalesced pull on scalar
                    nc.scalar.dma_start(
                        ytf[0:64, :].rearrange("p (s t) -> p s t", s=N_CORES),
                        a2a_out[0][:, 0:64, :].rearrange("s p t -> p s t"),
                    )
                else:
                    # per-slice pulls: the normalize/proj chain starts on
                    # slice 0 while the rest stream, instead of waiting for
                    # one monolithic transfer
                    for s in range(N_CORES):
                        nc.sync.dma_start(
                            ytf[64:128, s * 512:(s + 1) * 512],
                            a2a_out[1][s, 0:64, :],
                        )
                yield

            def norm_emit(h):
                # normalize head-half h of yT_full in place: broadcast the
                # recip rows across its 64 partitions, one multiply per s.
                # h=0 runs during the second collective's window.
                ones = (ones_h0, ones_h1)[h]
                r0 = 64 * h
                for s in range(N_CORES):
                    base = 64 * (s // 4)
                    col = (s % 4) * 512
                    pb = ps_main.tile([128, 512], F32, name="p_b", tag="ps")
                    nc.tensor.matmul(pb[:], ones[base:base + 1, :],
                                     rsb[h][base:base + 1, col:col + 512],
                                     start=True, stop=True)
                    ys = ytf[r0:r0 + 64, s * 512:(s + 1) * 512]
                    nc.vector.tensor_mul(ys, ys, pb[r0:r0 + 64, :])
                    if s % 4 == 3:
                        yield

            def finale_emit():
                # Fused h=1 normalize + projection, interleaved per source
                # block: the PE queue runs [pb(s), proj ct=s, pb(s+1), ...]
                # so each proj contraction step launches as soon as ITS
                # block is normalized, instead of all 8 normalizes queueing
                # ahead of the whole projection (which serialized the
                # finale behind the last ytf slice's pull+normalize).
                # PSUM budget: mt0/mt1 accumulate across the walk in ps_ss
                # ([128,1024] each); pb rotates in ps_main; mt2/mt3 trail
                # on the freed banks (~7us at the by-then-warm PE clock).
                def mm(dst, mt, n, ct):
                    nc.tensor.matmul(
                        dst,
                        ytf[:, ct * 512 + mt * 128:ct * 512 + (mt + 1) * 128],
                        wo_sb[:, ct * C + n * 512:ct * C + (n + 1) * 512],
                        start=(ct == 0),
                        stop=(ct == NCT - 1),
                    )

                def evac(mt, halves):
                    o_sb = wk.tile([128, C], BF16, name="o_sb", tag="o_sb")
                    for n in range(2):
                        nc.vector.tensor_copy(
                            o_sb[:, n * 512:(n + 1) * 512], halves[n])
                    nc.sync.dma_start(out[mt * 128:(mt + 1) * 128, :],
                                      o_sb[:])

                po01 = [ps_ss.tile([128, 1024], F32, name="p_oA", tag="pss")
                        for _ in range(2)]
                ones = ones_h1
                r0 = 64
                for s in range(N_CORES):
                    base = 64 * (s // 4)
                    col = (s % 4) * 512
                    pb = ps_main.tile([128, 512], F32, name="p_b", tag="ps")
                    nc.tensor.matmul(pb[:], ones[base:base + 1, :],
                                     rsb[1][base:base + 1, col:col + 512],
                                     start=True, stop=True)
                    ys = ytf[r0:r0 + 64, s * 512:(s + 1) * 512]
                    nc.vector.tensor_mul(ys, ys, pb[r0:r0 + 64, :])
                    # proj group s-1 (one step behind): its DVE normalize
                    # finished while pb(s) streamed, so the PE never waits
                    if s >= 1:
                        for mt in range(2):
                            for n in range(2):
                                mm(po01[mt][:, n * 512:(n + 1) * 512],
                                   mt, n, s - 1)
                    if s % 2:
                        yield
                for mt in range(2):
                    for n in range(2):
                        mm(po01[mt][:, n * 512:(n + 1) * 512],
                           mt, n, N_CORES - 1)
                # mt2 on the banks freed by the pb rotation
                po2 = [ps_main.tile([128, 512], F32, name="p_oB", tag="ps")
                       for _ in range(2)]
                for ct in range(NCT):
                    for n in range(2):
                        mm(po2[n][:], 2, n, ct)
                yield
                # evacuate mt0 before po3 reuses its PSUM buffer (pool
                # rotation: 3rd "pss" allocation lands on po01[0]'s banks)
                evac(0, [po01[0][:, n * 512:(n + 1) * 512] for n in range(2)])
                yield
                po3 = ps_ss.tile([128, 1024], F32, name="p_oA", tag="pss")
                for ct in range(NCT):
                    for n in range(2):
                        mm(po3[:, n * 512:(n + 1) * 512], 3, n, ct)
                yield
                evac(1, [po01[1][:, n * 512:(n + 1) * 512] for n in range(2)])
                yield
                evac(2, [po2[n][:] for n in range(2)])
                yield
                evac(3, [po3[:, n * 512:(n + 1) * 512] for n in range(2)])

            # ---------------- emission schedule ----------------
            _drain(qkv_emit(0))
            _drain(attn_emit(0, 0), qkv_emit(1))
            _drain(attn_emit(0, 1), wo_emit())
            nc.gpsimd.collective_compute(
                "AllToAll", mybir.AluOpType.bypass,
                replica_groups=[list(range(N_CORES))],
                ins=[a2a_in[0].opt().bitcast(F32)],
                outs=[a2a_out[0].opt().bitcast(F32)],
            )
            _drain(attn_emit(1, 0))
            _drain(attn_emit(1, 1))
            # The finale is pinned late via tile_wait_until: the scheduler
            # otherwise hoists these collective-gated instructions (ytf/rsb
            # pulls, pb broadcast matmuls) ahead of attn(1,*) work in the
            # in-order engine queues, stalling the PE ~50us on A2A#1.
            with tc.tile_wait_until(0.5):
                _drain(ytf_emit(0))
            nc.gpsimd.collective_compute(
                "AllToAll", mybir.AluOpType.bypass,
                replica_groups=[list(range(N_CORES))],
                ins=[a2a_in[1].opt().bitcast(F32)],
                outs=[a2a_out[1].opt().bitcast(F32)],
            )
            with tc.tile_wait_until(0.5):
                _drain(norm_emit(0))
            with tc.tile_wait_until(0.55):
                _drain(ytf_emit(1))
                _drain(finale_emit())

    nc.compile()
    return nc


def _host_inputs(x, tok_mask, Wqkv, Wproj, apply_pad_mask):
    bf = ml_dtypes.bfloat16
    x = np.ascontiguousarray(np.asarray(x, dtype=np.float32))
    Wqkv = np.ascontiguousarray(np.asarray(Wqkv, dtype=np.float32))
    Wproj = np.ascontiguousarray(np.asarray(Wproj, dtype=np.float32))
    xT = np.ascontiguousarray(
        np.concatenate([x[b].T for b in range(B)], axis=1)).astype(bf)
    wo_b = Wproj.astype(bf)
    r = np.arange(128)
    tri = (r[None, :] >= r[:, None]).astype(np.float32)  # keep if col >= row
    ident = np.eye(128, dtype=np.float32).astype(bf)
    if apply_pad_mask:
        padk = np.zeros((128, B * NK), np.float32)
        for b in range(B):
            padk[:, b * NK:(b + 1) * NK] = (
                np.asarray(tok_mask[b]).reshape(NK, 128).T.astype(np.float32)
            )
    else:
        padk = np.ones((128, B * NK), np.float32)

    in_maps = []
    for core in range(N_CORES):
        cols = slice(core * HL * D, (core + 1) * HL * D)
        # prepack into SBUF layout [p, which, ct, f]: each projection's
        # slice loads as one wide-row 2D DMA
        parts = []
        for which in range(3):
            w = Wqkv[:, which * C:(which + 1) * C][:, cols]  # [1024, 128]
            parts.append(
                w.reshape(NCT, 128, 128).transpose(1, 0, 2).reshape(128, -1))
        wqkv_c = np.ascontiguousarray(
            np.concatenate(parts, axis=1)).astype(bf)
        in_maps.append(
            {
                "xT": xT,
                "wqkv": wqkv_c,
                "wo": wo_b,
                "tri": tri,
                "ident": ident,
                "padk": padk,
            }
        )
    return in_maps


def kernel(x, tok_mask, Wqkv, Wproj, _run_kwargs=None):
    tok = np.asarray(tok_mask)
    apply_pad_mask = not bool(tok.all())
    key = apply_pad_mask
    if key not in _BUILD_CACHE:
        _BUILD_CACHE[key] = build_kernel(apply_pad_mask)
    nc = _BUILD_CACHE[key]
    in_maps = _host_inputs(x, tok_mask, Wqkv, Wproj, apply_pad_mask)
    kw = dict(_run_kwargs or {})
    res = bass_utils.run_bass_kernel_spmd(
        nc, in_maps, core_ids=list(range(N_CORES)), **kw
    )
    out = np.empty((B, T, C), np.float32)
    for core in range(N_CORES):
        b, jj = divmod(core, NQ)
        out[b, jj * 512:(jj + 1) * 512, :] = np.asarray(
            res.results[core]["out"], dtype=np.float32)
    kernel.last_result = res
    return out

